# revision 36
# baseline (speedup 1.0000x reference)
"""ExtractTensorPatches kernel for 8 trn2 NeuronCores.

Problem: x (4, 32, 256, 256) f32 -> out (4, 961, 32, 16, 16) f32 with
  out[b, ho*31+wo, c, i, j] = x[b, c, 8*ho+i, 8*wo+j] + EPS * patchsum
  patchsum = sum over the 16x16 patch at (8*ho, 8*wo).

Sharding: pure data parallelism over channels. Core k handles channels
[4k, 4k+4) for all 4 batches. Host gathers + permutes during unshard.

Per-core scheme (one tile set per batch b), row-deduplicated:
  X    [128, 2048]: partition p=(c, r8) holds its 8 UNIQUE rows
       (8*r8..8*r8+7) of channel c -> 4MB/core of loads, one HWDGE DMA
       per batch (8KB-contiguous per partition, spread over 16 engines).
  R2   [128, 31]: per-partition half-patch sums (DVE reduce).
  S    = R2 + shift(R2, +1 partition); Sp = shift(S, -1 partition) —
       the only cross-partition coupling, done with two tiny (15KB)
       SBUF->SBUF HWDGE DMAs + one DVE add.
  OUT  [128, 7936]: free = (half, wo, i_loc, j). half 0 = patch rows
       i=i_loc of ho=r8 (uses S); half 1 = rows i=8+i_loc of ho=r8-1
       (uses Sp). Both halves read only the partition's own rows via the
       overlapping-window AP in one fused scalar_tensor_tensor per row.
  stores: per-core DRAM layout (B, C_loc, r8, half, wo, i_loc, j) keeps
       each partition-half one contiguous 15872B chunk; per (half, c,
       r8-half) SWDGE DMAs (~250KB, one SDMA engine each via the per-DMA
       round-robin; dummy 4B DMAs steer early waves off load engines).
       Host reassembles (ho, i) from (r8, half, i_loc) for free.
"""
import sys

for _p in ("/opt/trn_rl_repo", "/root/.axon_site/_ro/trn_rl_repo"):
    if _p not in sys.path:
        sys.path.append(_p)

import numpy as np

B, C, H, W = 4, 32, 256, 256
WIN, STR = 16, 8
HO = (H - WIN) // STR + 1  # 31
L = HO * HO  # 961
EPS = 1e-6
NCORES = 8
CLOC = C // NCORES  # 4 channels per core
NP_PART = HO * CLOC  # 124 partitions in use

_nc_cache = {}


def _mk(t, dims):
    """Build a custom AP on a pool tile: partition dim + given free dims."""
    import concourse.bass as bass

    pstep = 1
    for d in t.tensor.shape[1:]:
        pstep *= d
    return bass.AP(t.tensor, t.offset, [[pstep, t.shape[0]]] + [list(d) for d in dims])


def build_nc():
    import concourse.bacc as bacc
    import concourse.mybir as mybir
    import concourse.tile as tile

    f32 = mybir.dt.float32
    nc = bacc.Bacc(
        "TRN2", target_bir_lowering=False, debug=False, num_devices=NCORES
    )
    x = nc.dram_tensor("x", [B, CLOC, H, W], f32, kind="ExternalInput").ap()
    # per-core layout (B, C_loc, ho, wo, i, j): each SBUF partition's
    # store is one fully-contiguous 31744B DRAM chunk (host permutes
    # back to (B, L, C, i, j) during the unshard gather).
    out = nc.dram_tensor(
        "out", [B, CLOC, 32, 2, HO, 8, WIN], f32, kind="ExternalOutput"
    ).ap()
    import concourse.bass as bass

    # SWDGE round-robin engine pointer: each gpsimd dma_start lands fully
    # on the next SDMA engine (mod 16). Loads run on HWDGE engines 0-3
    # (partition//32), so steer stores onto engines 4-15 with tiny dummy
    # DMAs that burn pointer slots 0-3.
    swdge_ptr = [0]
    dummy_dram = nc.dram_tensor("rr_align", [16, 1], f32).ap()

    with tile.TileContext(nc) as tc:
        with (
            tc.tile_pool(name="xin", bufs=4) as xpool,
            tc.tile_pool(name="stats", bufs=2) as spool,
            tc.tile_pool(name="outp", bufs=4) as opool,
        ):

            def align_store_group(OUT, lo, hi):
                while swdge_ptr[0] % 16 < 4:
                    k = swdge_ptr[0] % 16
                    nc.gpsimd.dma_start(
                        out=dummy_dram[k : k + 1, :], in_=OUT[0:1, lo : lo + 1]
                    )
                    swdge_ptr[0] += 1

            # ---- pass 0: all loads (issue at t=0) + stat-tile memsets
            Xs, R2l, R2sl, Sl, Spl = [], [], [], [], []
            for b in range(B):
                X = xpool.tile([128, 8 * W], f32, tag="X", name=f"X{b}")
                src = bass.AP(
                    x.tensor,
                    b * CLOC * H * W,
                    [[H * W, CLOC], [8 * W, 32], [1, 8 * W]],
                )
                nc.sync.dma_start(out=_mk(X, [[1, 8 * W]]), in_=src)
                Xs.append(X)
                R2 = spool.tile([128, HO], f32, tag=f"R2_{b}", name=f"R2{b}")
                R2s = spool.tile([128, HO], f32, tag=f"R2s_{b}", name=f"R2s{b}")
                S = spool.tile([128, HO], f32, tag=f"S_{b}", name=f"S{b}")
                Sp = spool.tile([128, HO], f32, tag=f"Sp_{b}", name=f"Sp{b}")
                nc.vector.memset(R2s[:, :], 0.0)
                nc.vector.memset(Sp[:, :], 0.0)
                R2l.append(R2); R2sl.append(R2s); Sl.append(S); Spl.append(Sp)

            # ---- pass 1: patch-sum chains, software-pipelined so each
            # tiny s2s shift-DMA's latency hides under the next batch's
            # reduce instead of stalling the in-order vector engine.
            def emit_R2(b):
                nc.vector.reduce_sum(
                    out=_mk(R2l[b], [[1, HO]]),
                    in_=_mk(Xs[b], [[STR, HO], [W, 8], [1, WIN]]),
                    axis=mybir.AxisListType.XY,
                )
                nc.sync.dma_start(out=R2sl[b][0:127, :], in_=R2l[b][1:128, :])

            def emit_S(b):
                nc.vector.tensor_add(Sl[b][:, :], R2l[b][:, :], R2sl[b][:, :])
                nc.sync.dma_start(out=Spl[b][1:128, :], in_=Sl[b][0:127, :])

            emit_R2(0); emit_R2(1); emit_S(0); emit_R2(2)
            emit_S(1); emit_R2(3); emit_S(2); emit_S(3)

            # ---- pass 2: fused add + stores per batch (dense DVE stream)
            for b in range(B):
                X, S, Sp = Xs[b], Sl[b], Spl[b]
                OUT = opool.tile([128, 2 * HO * 8 * WIN], f32, tag="OUT")
                opstep = 1
                for d in OUT.tensor.shape[1:]:
                    opstep *= d
                xpstep = 1
                for d in X.tensor.shape[1:]:
                    xpstep *= d
                hsz = HO * 8 * WIN  # 3968
                for h, Stile in ((0, S), (1, Sp)):
                    for il in range(8):
                        out_ap = bass.AP(
                            OUT.tensor,
                            OUT.offset + h * hsz + il * WIN,
                            [[opstep, 128], [8 * WIN, HO], [1, WIN]],
                        )
                        in1_ap = bass.AP(
                            X.tensor,
                            X.offset + il * W,
                            [[xpstep, 128], [STR, HO], [1, WIN]],
                        )
                        nc.vector.scalar_tensor_tensor(
                            out=out_ap,
                            in0=_mk(Stile, [[1, HO], [0, WIN]]),
                            scalar=float(EPS),
                            in1=in1_ap,
                            op0=mybir.AluOpType.mult,
                            op1=mybir.AluOpType.add,
                        )

                    align_store_group(OUT, h * hsz, (h + 1) * hsz)
                    for c in range(CLOC):
                        for r0 in (0, 16):
                            dst = bass.AP(
                                out.tensor,
                                ((b * CLOC + c) * 32 + r0) * 2 * hsz + h * hsz,
                                [[2 * hsz, 16], [1, hsz]],
                            )
                            nc.gpsimd.dma_start(
                                out=dst,
                                in_=OUT[
                                    c * 32 + r0 : c * 32 + r0 + 16,
                                    h * hsz : (h + 1) * hsz,
                                ],
                            )
                            swdge_ptr[0] += 1

    nc.compile()
    return nc


def get_nc():
    if "nc" not in _nc_cache:
        _nc_cache["nc"] = build_nc()
    return _nc_cache["nc"]


def kernel(x: np.ndarray) -> np.ndarray:
    from concourse.bass_utils import run_bass_kernel_spmd

    x = np.ascontiguousarray(np.asarray(x, dtype=np.float32))
    nc = get_nc()
    in_maps = [
        {"x": np.ascontiguousarray(x[:, k * CLOC : (k + 1) * CLOC])}
        for k in range(NCORES)
    ]
    res = run_bass_kernel_spmd(nc, in_maps, list(range(NCORES)))
    # res[k]["out"]: (B, CLOC, r8=32, half=2, wo, i_loc=8, j).
    # Patch row block i<8 lives at (r8=ho, half0); i>=8 at (r8=ho+1, half1).
    arr = np.stack([r["out"] for r in res.results], axis=0)
    own = arr[:, :, :, 0:31, 0]  # (k, B, CLOC, ho, wo, 8, 16)
    prv = arr[:, :, :, 1:32, 1]
    comb = np.concatenate([own, prv], axis=5)  # i dim -> 16
    return np.ascontiguousarray(
        comb.transpose(1, 3, 4, 0, 2, 5, 6).reshape(B, L, C, WIN, WIN)
    )


# revision 37
# speedup vs baseline: 1.0315x; 1.0315x over previous
"""ExtractTensorPatches kernel for 8 trn2 NeuronCores.

Problem: x (4, 32, 256, 256) f32 -> out (4, 961, 32, 16, 16) f32 with
  out[b, ho*31+wo, c, i, j] = x[b, c, 8*ho+i, 8*wo+j] + EPS * patchsum
  patchsum = sum over the 16x16 patch at (8*ho, 8*wo).

Sharding: pure data parallelism over channels. Core k handles channels
[4k, 4k+4) for all 4 batches. Host gathers + permutes during unshard.

Per-core scheme (one tile set per batch b), row-deduplicated:
  X    [128, 2048]: partition p=(c, r8) holds its 8 UNIQUE rows
       (8*r8..8*r8+7) of channel c -> 4MB/core of loads, one HWDGE DMA
       per batch (8KB-contiguous per partition, spread over 16 engines).
  R2   [128, 31]: per-partition half-patch sums (DVE reduce).
  S    = R2 + shift(R2, +1 partition); Sp = shift(S, -1 partition) —
       the only cross-partition coupling, done with two tiny (15KB)
       SBUF->SBUF HWDGE DMAs + one DVE add.
  OUT  [128, 7936]: free = (half, wo, i_loc, j). half 0 = patch rows
       i=i_loc of ho=r8 (uses S); half 1 = rows i=8+i_loc of ho=r8-1
       (uses Sp). Both halves read only the partition's own rows via the
       overlapping-window AP in one fused scalar_tensor_tensor per row.
  stores: per-core DRAM layout (B, C_loc, r8, half, wo, i_loc, j) keeps
       each partition-half one contiguous 15872B chunk; per (half, c,
       r8-half) SWDGE DMAs (~250KB, one SDMA engine each via the per-DMA
       round-robin; dummy 4B DMAs steer early waves off load engines).
       Host reassembles (ho, i) from (r8, half, i_loc) for free.
"""
import sys

for _p in ("/opt/trn_rl_repo", "/root/.axon_site/_ro/trn_rl_repo"):
    if _p not in sys.path:
        sys.path.append(_p)

import numpy as np

B, C, H, W = 4, 32, 256, 256
WIN, STR = 16, 8
HO = (H - WIN) // STR + 1  # 31
L = HO * HO  # 961
EPS = 1e-6
NCORES = 8
CLOC = C // NCORES  # 4 channels per core
NP_PART = HO * CLOC  # 124 partitions in use

_nc_cache = {}


def _mk(t, dims):
    """Build a custom AP on a pool tile: partition dim + given free dims."""
    import concourse.bass as bass

    pstep = 1
    for d in t.tensor.shape[1:]:
        pstep *= d
    return bass.AP(t.tensor, t.offset, [[pstep, t.shape[0]]] + [list(d) for d in dims])


def build_nc():
    import concourse.bacc as bacc
    import concourse.mybir as mybir
    import concourse.tile as tile

    f32 = mybir.dt.float32
    nc = bacc.Bacc(
        "TRN2", target_bir_lowering=False, debug=False, num_devices=NCORES
    )
    x = nc.dram_tensor("x", [B, CLOC, H, W], f32, kind="ExternalInput").ap()
    # per-core layout (B, C_loc, ho, wo, i, j): each SBUF partition's
    # store is one fully-contiguous 31744B DRAM chunk (host permutes
    # back to (B, L, C, i, j) during the unshard gather).
    out = nc.dram_tensor(
        "out", [B, CLOC, 32, 2, HO, 8, WIN], f32, kind="ExternalOutput"
    ).ap()
    import concourse.bass as bass

    # SWDGE round-robin engine pointer: each gpsimd dma_start lands fully
    # on the next SDMA engine (mod 16). Loads run on HWDGE engines 0-3
    # (partition//32), so steer stores onto engines 4-15 with tiny dummy
    # DMAs that burn pointer slots 0-3.
    swdge_ptr = [0]
    dummy_dram = nc.dram_tensor("rr_align", [16, 1], f32).ap()

    with tile.TileContext(nc) as tc:
        with (
            tc.tile_pool(name="xin", bufs=4) as xpool,
            tc.tile_pool(name="stats", bufs=2) as spool,
            tc.tile_pool(name="outp", bufs=3) as opool,
        ):

            def align_store_group(OUT, lo, hi):
                while swdge_ptr[0] % 16 < 4:
                    k = swdge_ptr[0] % 16
                    nc.gpsimd.dma_start(
                        out=dummy_dram[k : k + 1, :], in_=OUT[0:1, lo : lo + 1]
                    )
                    swdge_ptr[0] += 1

            # Stat-chain helpers; 1-deep software pipeline: the next
            # batch's R2 reduce and S add are slotted inside this batch's
            # two stt halves, hiding the s2s shift-DMA latencies.
            st = {}

            def emit_load(b):
                X = xpool.tile([128, 8 * W], f32, tag="X", name=f"X{b}")
                src = bass.AP(
                    x.tensor,
                    b * CLOC * H * W,
                    [[H * W, CLOC], [8 * W, 32], [1, 8 * W]],
                )
                nc.sync.dma_start(out=_mk(X, [[1, 8 * W]]), in_=src)
                st[b] = [X]

            def emit_R2(b):
                X = st[b][0]
                R2 = spool.tile([128, HO], f32, tag="R2", name=f"R2{b}")
                R2s = spool.tile([128, HO], f32, tag="R2s", name=f"R2s{b}")
                nc.vector.memset(R2s[:, :], 0.0)
                nc.vector.reduce_sum(
                    out=_mk(R2, [[1, HO]]),
                    in_=_mk(X, [[STR, HO], [W, 8], [1, WIN]]),
                    axis=mybir.AxisListType.XY,
                )
                nc.sync.dma_start(out=R2s[0:127, :], in_=R2[1:128, :])
                st[b] += [R2, R2s]

            def emit_S(b):
                _, R2, R2s = st[b]
                S = spool.tile([128, HO], f32, tag="S", name=f"S{b}")
                Sp = spool.tile([128, HO], f32, tag="Sp", name=f"Sp{b}")
                nc.vector.memset(Sp[:, :], 0.0)
                nc.vector.tensor_add(S[:, :], R2[:, :], R2s[:, :])
                nc.sync.dma_start(out=Sp[1:128, :], in_=S[0:127, :])
                st[b] += [S, Sp]

            for b in range(B):
                emit_load(b)
            emit_R2(0)
            emit_S(0)
            for b in range(B):
                X = st[b][0]
                S, Sp = st[b][3], st[b][4]
                OUT = opool.tile([128, 2 * HO * 8 * WIN], f32, tag="OUT")
                opstep = 1
                for d in OUT.tensor.shape[1:]:
                    opstep *= d
                xpstep = 1
                for d in X.tensor.shape[1:]:
                    xpstep *= d
                hsz = HO * 8 * WIN  # 3968
                for h, Stile in ((0, S), (1, Sp)):
                    if h == 0 and b + 1 < B:
                        emit_R2(b + 1)
                    for il in range(8):
                        out_ap = bass.AP(
                            OUT.tensor,
                            OUT.offset + h * hsz + il * WIN,
                            [[opstep, 128], [8 * WIN, HO], [1, WIN]],
                        )
                        in1_ap = bass.AP(
                            X.tensor,
                            X.offset + il * W,
                            [[xpstep, 128], [STR, HO], [1, WIN]],
                        )
                        nc.vector.scalar_tensor_tensor(
                            out=out_ap,
                            in0=_mk(Stile, [[1, HO], [0, WIN]]),
                            scalar=float(EPS),
                            in1=in1_ap,
                            op0=mybir.AluOpType.mult,
                            op1=mybir.AluOpType.add,
                        )
                    if h == 0 and b + 1 < B:
                        emit_S(b + 1)

                    align_store_group(OUT, h * hsz, (h + 1) * hsz)
                    for c in range(CLOC):
                        for r0 in (0, 16):
                            dst = bass.AP(
                                out.tensor,
                                ((b * CLOC + c) * 32 + r0) * 2 * hsz + h * hsz,
                                [[2 * hsz, 16], [1, hsz]],
                            )
                            nc.gpsimd.dma_start(
                                out=dst,
                                in_=OUT[
                                    c * 32 + r0 : c * 32 + r0 + 16,
                                    h * hsz : (h + 1) * hsz,
                                ],
                            )
                            swdge_ptr[0] += 1

    nc.compile()
    return nc


def get_nc():
    if "nc" not in _nc_cache:
        _nc_cache["nc"] = build_nc()
    return _nc_cache["nc"]


def kernel(x: np.ndarray) -> np.ndarray:
    from concourse.bass_utils import run_bass_kernel_spmd

    x = np.ascontiguousarray(np.asarray(x, dtype=np.float32))
    nc = get_nc()
    in_maps = [
        {"x": np.ascontiguousarray(x[:, k * CLOC : (k + 1) * CLOC])}
        for k in range(NCORES)
    ]
    res = run_bass_kernel_spmd(nc, in_maps, list(range(NCORES)))
    # res[k]["out"]: (B, CLOC, r8=32, half=2, wo, i_loc=8, j).
    # Patch row block i<8 lives at (r8=ho, half0); i>=8 at (r8=ho+1, half1).
    arr = np.stack([r["out"] for r in res.results], axis=0)
    own = arr[:, :, :, 0:31, 0]  # (k, B, CLOC, ho, wo, 8, 16)
    prv = arr[:, :, :, 1:32, 1]
    comb = np.concatenate([own, prv], axis=5)  # i dim -> 16
    return np.ascontiguousarray(
        comb.transpose(1, 3, 4, 0, 2, 5, 6).reshape(B, L, C, WIN, WIN)
    )
